# revision 2
# baseline (speedup 1.0000x reference)
"""Modulated deformable conv (DCNv2) Trainium2 Bass kernel, v3.

Sharding: 8 cores = 4 batches x 2 pixel-halves (image rows 0-63 / 64-127).
Key changes vs v1:
  - Host preps xdup[y, x, r, c] = x[y+r, x, c] (row-pair duplicated layout),
    so ONE dma_gather per tap fetches all 4 bilinear corners (1KB elems):
    gather count drops 2x, elements are 1KB contiguous.
  - Bilinear weights via clamp+relu-hat: q = clamp(floor(py), 0, 126),
    wA_g = relu(1 - |py - (q+g)|); handles all borders with no masks.
  - Weight rows transposed back on the PE and staged to DRAM with one
    contiguous 576KB write (v1 did 6 heavily-strided DMAs).
  - Matmuls at free-dim 2048 (fewer instructions).
"""

import numpy as np

import concourse.bass as bass
import concourse.tile as tile
from concourse import bacc, mybir
from concourse.bass_utils import run_bass_kernel_spmd
from concourse.masks import make_identity

f16 = mybir.dt.float16
f32 = mybir.dt.float32
i16 = mybir.dt.int16
i32 = mybir.dt.int32
Alu = mybir.AluOpType
Act = mybir.ActivationFunctionType

H = W = 128
HW = H * W
C = 128
O = 128
K = 9
NCH = 27          # conv output channels: [off_y(9), off_x(9), mask_logit(9)]
NPX = HW // 2     # pixels per core (one half: 64 image rows)
BLK = NPX // 128  # 64 local row-blocks
CHUNK = 2048      # pixels per PSUM pass
NCHUNK = NPX // CHUNK
NS = NPX // 16    # index slots per row in dma_gather wrap layout


def _ap(src_ap, offset, pattern):
    return bass.AP(tensor=src_ap.tensor, offset=src_ap.offset + offset,
                   ap=[list(p) for p in pattern])


def _apf(src_ap, offset, free_pattern):
    return bass.AP(tensor=src_ap.tensor, offset=src_ap.offset + offset,
                   ap=[list(src_ap.ap[0])] + [list(p) for p in free_pattern])


def build_kernel(debug=False):
    nc = bacc.Bacc("TRN2", target_bir_lowering=False, debug=False,
                   enable_asserts=True)

    # ---- I/O ----
    xpad_in = nc.dram_tensor("xpad", [C, 66 * 130], f16, kind="ExternalInput")
    xdup_in = nc.dram_tensor("xdup", [HW * 2 * C], f16, kind="ExternalInput")
    wconv_in = nc.dram_tensor("wconv", [C, K * NCH], f16, kind="ExternalInput")
    bias_in = nc.dram_tensor("bias", [NCH, 1], f32, kind="ExternalInput")
    w2_in = nc.dram_tensor("w2", [C, K * O], f16, kind="ExternalInput")
    basey_in = nc.dram_tensor("basey", [128, K * BLK], f32, kind="ExternalInput")
    basex_in = nc.dram_tensor("basex", [128, K], f32, kind="ExternalInput")
    out_o = nc.dram_tensor("out", [O, NPX], f32, kind="ExternalOutput")

    wrows_d = nc.dram_tensor("wrows_d", [4 * K * NPX], f16)  # [k, j, px]
    idx_d = nc.dram_tensor("idx_d", [K * NPX], i16)          # [k, px]

    if debug:
        dbg_conv = nc.dram_tensor("dbg_conv", [NCH, NPX], f32, kind="ExternalOutput")
        dbg_wt = nc.dram_tensor("dbg_wt", [4 * K, NPX], f16, kind="ExternalOutput")
        dbg_idx = nc.dram_tensor("dbg_idx", [K, NPX], i16, kind="ExternalOutput")
        dbg_g = nc.dram_tensor("dbg_g", [128, 4 * CHUNK], f16, kind="ExternalOutput")

    with tile.TileContext(nc) as tc:
        with tc.tile_pool(name="persist", bufs=1) as persist:
            w2_t = persist.tile([C, K, O], f16)
            nc.sync.dma_start(w2_t[:], w2_in.ap())
            idx_sb = persist.tile([128, K, NS], i16)

            # ================= Phase B: offset/mask convs =================
            with tc.tile_pool(name="convph", bufs=1) as cph:
                xpad_t = cph.tile([C, 66, 130], f16)
                nc.sync.dma_start(xpad_t[:], xpad_in.ap())
                wconv_t = cph.tile([C, K, NCH], f16)
                nc.sync.dma_start(wconv_t[:], wconv_in.ap())
                bias_t = cph.tile([NCH, 1], f32)
                nc.sync.dma_start(bias_t[:], bias_in.ap())
                conv_sb = cph.tile([NCH, NPX], f32)
                identf = cph.tile([128, 128], f32)
                make_identity(nc, identf[:])
                identh = cph.tile([128, 128], f16)
                make_identity(nc, identh[:])

                with tc.tile_pool(name="psconv", bufs=4, space="PSUM") as psc:
                    for t in range(NPX // 512):  # 16 tiles of 512 px (4 rows)
                        ps = psc.tile([NCH, 512], f32)
                        for k in range(K):
                            ki, kj = k // 3, k % 3
                            rhs = _apf(xpad_t[:], (t * 4 + ki) * 130 + kj,
                                       [[130, 4], [1, 128]])
                            nc.tensor.matmul(ps[:], wconv_t[:, k, :], rhs,
                                             start=(k == 0), stop=(k == K - 1))
                        nc.scalar.activation(conv_sb[:, t * 512:(t + 1) * 512],
                                             ps[:], Act.Identity,
                                             bias=bias_t[:, 0:1])
                if debug:
                    nc.sync.dma_start(dbg_conv.ap(), conv_sb[:])

                # ========= Phase C: transpose + weight/index math =========
                with tc.tile_pool(name="wmath", bufs=1) as wm:
                    offs = wm.tile([128, NCH, BLK], f32)
                    with tc.tile_pool(name="pst", bufs=2, space="PSUM") as pst:
                        for grp in range(BLK // 16):
                            ps = pst.tile([128, 16 * NCH], f32)
                            for j in range(16):
                                blk = grp * 16 + j
                                nc.tensor.transpose(
                                    ps[:, j * NCH:(j + 1) * NCH],
                                    conv_sb[:, blk * 128:(blk + 1) * 128],
                                    identf[0:NCH, 0:NCH])
                            src = _apf(ps[:], 0, [[1, NCH], [NCH, 16]])
                            dst = _apf(offs[:], grp * 16, [[BLK, NCH], [1, 16]])
                            nc.scalar.activation(dst, src, Act.Copy)

                    basey_t = wm.tile([128, K, BLK], f32)
                    nc.sync.dma_start(basey_t[:], basey_in.ap())
                    basex_t = wm.tile([128, K], f32)
                    nc.sync.dma_start(basex_t[:], basex_in.ap())

                    shp = [128, K, BLK]

                    def scratch(tag):
                        return wm.tile(shp, f32, tag=tag, name="sc_" + tag)

                    off_y = offs[:, 0:K, :]
                    off_x = offs[:, K:2 * K, :]
                    logits = offs[:, 2 * K:3 * K, :]

                    py = scratch("py")
                    nc.vector.tensor_tensor(py[:], off_y, basey_t[:], Alu.add)
                    px = scratch("px")
                    bx_b = basex_t[:, :, None].to_broadcast(tuple(shp))
                    nc.vector.tensor_tensor(px[:], off_x, bx_b, Alu.add)

                    def hats(pv, tagp):
                        """q = clamp(floor(pv), 0, 126); w0/w1 = relu-hat."""
                        ri = wm.tile(shp, i32, tag="ri", name="ri")
                        nc.vector.tensor_copy(ri[:], pv[:])
                        rf = scratch(tagp + "rf")
                        nc.vector.tensor_copy(rf[:], ri[:])
                        gt_ = scratch(tagp + "gt")
                        nc.vector.tensor_tensor(gt_[:], rf[:], pv[:], Alu.is_gt)
                        q = scratch(tagp + "q")
                        nc.vector.tensor_tensor(q[:], rf[:], gt_[:], Alu.subtract)
                        nc.vector.tensor_scalar(q[:], q[:], 0.0, 126.0,
                                                Alu.max, Alu.min)
                        t = scratch(tagp + "t")
                        nc.vector.tensor_tensor(t[:], pv[:], q[:], Alu.subtract)
                        a = scratch(tagp + "a")
                        nc.vector.tensor_scalar(a[:], t[:], -1.0, 1.0,
                                                Alu.mult, Alu.add)   # 1-t
                        b = scratch(tagp + "b")
                        nc.vector.tensor_scalar(b[:], t[:], 1.0, None,
                                                Alu.add)             # 1+t
                        w0 = scratch(tagp + "w0")
                        nc.vector.tensor_tensor(w0[:], a[:], b[:], Alu.min)
                        nc.scalar.activation(w0[:], w0[:], Act.Relu)
                        c = scratch(tagp + "c")
                        nc.vector.tensor_scalar(c[:], t[:], -1.0, 2.0,
                                                Alu.mult, Alu.add)   # 2-t
                        w1 = scratch(tagp + "w1")
                        nc.vector.tensor_tensor(w1[:], c[:], t[:], Alu.min)
                        nc.scalar.activation(w1[:], w1[:], Act.Relu)
                        return q, w0, w1

                    q, wA0, wA1 = hats(py, "y")
                    bx, wB0, wB1 = hats(px, "x")

                    msig = scratch("msig")
                    nc.scalar.activation(msig[:], logits, Act.Sigmoid)
                    A0m = scratch("A0m")
                    nc.vector.tensor_tensor(A0m[:], wA0[:], msig[:], Alu.mult)
                    A1m = scratch("A1m")
                    nc.vector.tensor_tensor(A1m[:], wA1[:], msig[:], Alu.mult)

                    # weight maps, slot order j = s*2 + g
                    wtpp = wm.tile([128, K, 4, BLK], f16)
                    for s, Ws in ((0, wB0), (1, wB1)):
                        for g, Ag in ((0, A0m), (1, A1m)):
                            nc.vector.tensor_tensor(wtpp[:, :, s * 2 + g, :],
                                                    Ws[:], Ag[:], Alu.mult)

                    # gather token index = q*128 + bx
                    idxf = scratch("idxf")
                    nc.vector.tensor_scalar(idxf[:], q[:], 128.0, None, Alu.mult)
                    nc.vector.tensor_tensor(idxf[:], idxf[:], bx[:], Alu.add)
                    idx16 = wm.tile([128, K, BLK], i16)
                    nc.vector.tensor_copy(idx16[:], idxf[:])

                    # ---- weight rows: PE-transpose wtpp -> [36, NPX] ----
                    # row value wrows[(k,j), i] with i = pp*64 + blk
                    wrows_sb = wm.tile([36, NPX], f16)
                    with tc.tile_pool(name="ptw", bufs=2, space="PSUM") as ptw:
                        for grp in range(BLK // 16):
                            pw = ptw.tile([36, 16 * 128], f16)
                            for j in range(16):
                                blk = grp * 16 + j
                                src = _apf(wtpp[:], blk, [[BLK, K * 4]])
                                nc.tensor.transpose(
                                    pw[:, j * 128:(j + 1) * 128],
                                    src, identh[:])
                            # pw[ch36, j*128 + pp] -> wrows_sb[ch36, pp*64+grp*16+j]
                            dst = _apf(wrows_sb[:], grp * 16,
                                       [[BLK, 128], [1, 16]])
                            srcp = _apf(pw[:], 0, [[1, 128], [128, 16]])
                            nc.scalar.activation(dst, srcp, Act.Copy)
                    nc.sync.dma_start(wrows_d.ap(), wrows_sb[:])

                    # ---- stage idx -> DRAM (pp-major flat per tap) ----
                    nc.sync.dma_start(
                        _ap(idx_d.ap(), 0, [[BLK, 128], [NPX, K], [1, BLK]]),
                        idx16[:])

            # wrap indices to dma_gather layout: [16, K*NS] via one transpose
            nc.sync.dma_start_transpose(
                idx_sb[0:16, :, :].rearrange("p m s -> p (m s)"),
                _ap(idx_d.ap(), 0, [[16, K * NS], [1, 16]]))
            # log-doubling replication 16 -> 128 partitions (3 DMAs)
            for lo, n in ((16, 16), (32, 32), (64, 64)):
                nc.sync.dma_start(idx_sb[lo:lo + n, :, :], idx_sb[0:n, :, :])
            if debug:
                nc.sync.dma_start(dbg_wt.ap(),
                                  _ap(wrows_d.ap(), 0, [[NPX, 4 * K], [1, NPX]]))
                nc.sync.dma_start(dbg_idx.ap(),
                                  _ap(idx_d.ap(), 0, [[NPX, K], [1, NPX]]))

            # ============ Phase D: gather + weight + GEMM ============
            with tc.tile_pool(name="gath", bufs=3) as gp, \
                 tc.tile_pool(name="wrep", bufs=3) as wp, \
                 tc.tile_pool(name="wgt", bufs=2) as wgp, \
                 tc.tile_pool(name="oev", bufs=2) as op_, \
                 tc.tile_pool(name="psout", bufs=2, space="PSUM") as pso:
                for ch in range(NCHUNK):
                    ps = pso.tile([O, CHUNK], f32)
                    for k in range(K):
                        gt = gp.tile([128, 4, CHUNK], f16, tag="g")
                        in_ap = _ap(xdup_in.ap(), 0, [[256, HW - 1], [1, 512]])
                        out_ap = _apf(gt[:], 0, [[CHUNK, 4], [1, CHUNK]])
                        idxs = idx_sb[:, k,
                                      ch * (CHUNK // 16):(ch + 1) * (CHUNK // 16)]
                        nc.gpsimd.dma_gather(out_ap, in_ap, idxs,
                                             num_idxs=CHUNK,
                                             num_idxs_reg=CHUNK,
                                             elem_size=512, elem_step=256,
                                             transpose=True,
                                             single_packet=False)
                        if debug and ch == 0 and k == 0:
                            nc.sync.dma_start(dbg_g.ap(), gt[:])
                        wr = wp.tile([128, 4, CHUNK], f16, tag="w")
                        nc.sync.dma_start(
                            wr[:],
                            _ap(wrows_d.ap(), k * 4 * NPX + ch * CHUNK,
                                [[0, 128], [NPX, 4], [1, CHUNK]]))
                        wg = wgp.tile([128, 4, CHUNK], f16, tag="x")
                        nc.vector.tensor_tensor(wg[:], gt[:], wr[:], Alu.mult)
                        for j in range(4):
                            first = (k == 0 and j == 0)
                            last = (k == K - 1 and j == 3)
                            for b in range(CHUNK // 512):
                                nc.tensor.matmul(
                                    ps[:, b * 512:(b + 1) * 512],
                                    w2_t[:, k, :],
                                    wg[:, j, b * 512:(b + 1) * 512],
                                    start=first, stop=last)
                    ot = op_.tile([O, CHUNK], f32, tag="o")
                    nc.scalar.activation(ot[:], ps[:], Act.Copy)
                    nc.sync.dma_start(
                        _ap(out_o.ap(), ch * CHUNK, [[NPX, O], [1, CHUNK]]), ot[:])
    nc.compile()
    return nc


def _host_inputs(x, w_off, b_off, w_mod, b_mod, w_reg):
    """Build the 8 per-core input maps."""
    wcat = np.concatenate([w_off[0::2], w_off[1::2], w_mod], axis=0)
    bcat = np.concatenate([b_off[0::2], b_off[1::2], b_mod], axis=0)
    wconv = np.ascontiguousarray(
        wcat.transpose(1, 2, 3, 0).reshape(C, K * NCH)).astype(np.float16)
    bias = bcat.reshape(NCH, 1).astype(np.float32)
    w2 = np.ascontiguousarray(
        (w_reg * 2.0).transpose(1, 2, 3, 0).reshape(C, K * O)).astype(np.float16)
    ki = np.arange(K) // 3
    kj = np.arange(K) % 3
    basex = (np.arange(128)[:, None] + kj[None, :] - 1).astype(np.float32)

    B = x.shape[0]
    xdups = []
    for b in range(B):
        xt = x[b].transpose(1, 2, 0).astype(np.float16)  # [y, x, c]
        xd = np.zeros((H, W, 2, C), dtype=np.float16)
        xd[:, :, 0, :] = xt
        xd[:H - 1, :, 1, :] = xt[1:]
        xdups.append(np.ascontiguousarray(xd.reshape(HW * 2 * C)))

    maps = []
    for core in range(8):
        b, hf = core // 2, core % 2
        xpadfull = np.zeros((C, 130, 130), dtype=np.float16)
        xpadfull[:, 1:129, 1:129] = x[b].astype(np.float16)
        xpad = np.ascontiguousarray(xpadfull[:, 64 * hf:64 * hf + 66, :])
        rloc = 64 * hf + np.arange(BLK)
        basey = np.broadcast_to(
            (rloc[None, :] + ki[:, None] - 1)[None, :, :],
            (128, K, BLK)).reshape(128, K * BLK).astype(np.float32)
        maps.append({
            "xpad": xpad.reshape(C, 66 * 130),
            "xdup": xdups[b],
            "wconv": wconv,
            "bias": bias,
            "w2": w2,
            "basey": np.ascontiguousarray(basey),
            "basex": basex,
        })
    return maps


_NC_CACHE = {}


def kernel(x, w_off, b_off, w_mod, b_mod, w_reg, debug=False, trace=False):
    x = np.asarray(x)
    key = ("nc", debug)
    if key not in _NC_CACHE:
        _NC_CACHE[key] = build_kernel(debug=debug)
    nc = _NC_CACHE[key]
    maps = _host_inputs(x, np.asarray(w_off), np.asarray(b_off),
                        np.asarray(w_mod), np.asarray(b_mod), np.asarray(w_reg))
    res = run_bass_kernel_spmd(nc, maps, core_ids=list(range(8)), trace=trace)
    B = x.shape[0]
    out = np.empty((B, O, H, W), dtype=np.float32)
    for core in range(8):
        b, hf = core // 2, core % 2
        out[b, :, 64 * hf:64 * (hf + 1), :] = \
            res.results[core]["out"].reshape(O, 128, BLK).transpose(0, 2, 1)
    kernel._last_results = res
    return out
